# revision 24
# baseline (speedup 1.0000x reference)
"""GQA attention (B=2, S=2048, D=2048, H=16, KVH=4, DH=128) with RoPE and a
causal mask, distributed over 8 Trainium2 NeuronCores.

Sharding: 4 kv-head groups (tensor parallel) x 2 batch elements = 8 cores.
Each core computes its head group's Q/K/V projections, RoPE, attention, and a
partial output projection (the wo columns belonging to its heads). The host
sums the 4 partial outputs per batch element (no on-device collectives).

Layout tricks (all host-side, free):
  - Every matmul operand is passed pre-transposed/pre-arranged in its exact
    SBUF layout, so all DMAs are contiguous and no on-device transposes exist.
  - The head dim of wq/wk is permuted (even idxs then odd idxs) so RoPE's
    (real, imag) pairs become partition blocks [0:64) / [64:128) -> RoPE is 4
    vector ops per tile. Scores are invariant to this permutation since both
    q and k use it.
  - Scores are computed transposed (k on partitions, q on free axis) so the
    softmax denominator is a ones-matmul on the TensorEngine and P^T feeds
    the AV matmul directly; nothing is ever transposed on-device.
  - Softmax skips the max subtraction: inputs are well-scaled N(0,1)-ish and
    exp stays far from overflow in fp32.

Schedule (v2):
  - PSUM banks are hand-assigned via 8 singleton pool tags: projection wave
    groups use all 8; attention scores cycle banks 0-3, den bank 4, av bank
    5; the output projection double-buffers banks 6/7. Attention and oproj
    therefore never contend for PSUM, so oproj(qc) pipelines with the next
    qc's attention without stalling the PE.
  - Projections run as waves [K4+V4][V4][Qh0+V4][Qh1+V4][Qh2][Qh3] with each
    wave's group tails staggered (last 4 chunks group-major + rope emitted
    immediately) so the DVE rope drain overlaps the next groups' matmuls.
  - h0's attention blocks are interleaved between the Q waves (their inputs
    are ready wave-by-wave), bridging the projection->attention transition
    and keeping the PE HAM clock-gate warm.
  - Softmax denominator uses fp8(e4m3) DoubleRow matmuls on full score
    tiles (two k-tiles per pass; numerically safe because den sums positive
    values, verified ~0.7% total vs 0.45% all-bf16), diagonal tiles stay
    bf16 and causally trimmed.
  - Per attention block the PE order is scores -> AV -> den so the scalar
    engine's exp (and the DVE's fp8 casts) never stall the in-order PE queue.
  - DMA: x chunk 0 is issued as four column-quarter transfers so the first
    matmul starts ~3us earlier; wv/wk chunk-0 are tiny dedicated transfers;
    wq/cs/sn are interleaved into the x stream so the Q waves and ropes are
    never input-starved. Early qc output rows batch 4 copies into one
    [128,2048] DMA; the final qc copies alternate scalar/vector and DMA
    per-tile to minimize the drain tail.
"""

import numpy as np
import ml_dtypes

B, S, D = 2, 2048, 2048
H, KVH, DH = 16, 4, 128
G = KVH              # tensor-parallel head groups
HPG = H // KVH       # q heads per group
E = HPG * DH         # per-core q/attn dim (512)
DCH = D // 128       # d chunks of 128 (16)
SCH = S // 512       # s chunks of 512 (4)
STL = S // 128       # s tiles of 128 (16)
SCALE = float(1.0 / np.sqrt(DH))
BF16 = ml_dtypes.bfloat16
F8 = ml_dtypes.float8_e4m3
USE_DR = True  # fp8 DoubleRow denominator
USE_CAST = False  # emit fp8 casts even if DR disabled (debug)

_nc_cache = {}


def _install_profile_hook():
    """Register the axon NTFF profiling hook if the environment's antenv stub
    lacks it (best effort; only needed when tracing)."""
    try:
        import antenv.axon_hooks  # noqa: F401
        return
    except ImportError:
        pass
    try:
        import sys
        import types

        import antenv
        from trn_agent_boot.trn_boot import _ntff_profile_via_ctypes

        mod = types.ModuleType("antenv.axon_hooks")
        _store = {}
        mod.set_axon_ntff_profile_hook = lambda h: _store.__setitem__("h", h)
        mod.get_axon_ntff_profile_hook = lambda: _store.get("h")
        sys.modules["antenv.axon_hooks"] = mod
        antenv.axon_hooks = mod
        mod.set_axon_ntff_profile_hook(
            _ntff_profile_via_ctypes("/opt/axon/libaxon_pjrt.so")
        )
        import concourse.bass_utils as bu

        bu.upload_artifacts = lambda tmpdir: f"file://{tmpdir}"
    except Exception:
        pass


def _build(variant):
    """Build + compile the per-core kernel. variant: causal | nomask | generic."""
    import concourse.mybir as mybir
    import concourse.tile as tile
    from concourse import bacc

    fp32 = mybir.dt.float32
    bf16 = mybir.dt.bfloat16
    f8 = mybir.dt.float8e4
    EXP = mybir.ActivationFunctionType.Exp
    DR = mybir.MatmulPerfMode.DoubleRow

    nc = bacc.Bacc(None, target_bir_lowering=False, num_devices=8)

    xT = nc.declare_dram_parameter("xT", [128, DCH, S], bf16, isOutput=False)
    wqT = nc.declare_dram_parameter("wqT", [128, DCH, E], bf16, isOutput=False)
    wkT = nc.declare_dram_parameter("wkT", [128, DCH, DH], bf16, isOutput=False)
    wvT = nc.declare_dram_parameter("wvT", [128, DCH, DH], bf16, isOutput=False)
    woT = nc.declare_dram_parameter("woT", [128, HPG, D], bf16, isOutput=False)
    cs = nc.declare_dram_parameter("cs", [128, S], bf16, isOutput=False)
    sn = nc.declare_dram_parameter("sn", [128, S], bf16, isOutput=False)
    ones8 = nc.declare_dram_parameter("ones8", [128, 2, 128], f8, isOutput=False)
    if variant == "causal":
        cm = nc.declare_dram_parameter("ltri", [128, 128], bf16, isOutput=False)
    if variant == "generic":
        mT = nc.declare_dram_parameter("maskT", [128, STL, S], fp32, isOutput=False)
    outp = nc.declare_dram_parameter("out", [STL, 128, D], bf16, isOutput=True)

    with tile.TileContext(nc) as tc:
        with (
            tc.tile_pool(name="const", bufs=1) as cpool,
            tc.tile_pool(name="tmp", bufs=4) as tpool,
            tc.tile_pool(name="ptp", bufs=14) as ptpool,
            tc.tile_pool(name="p8p", bufs=8) as p8pool,
            tc.tile_pool(name="otp", bufs=2) as opool,
            tc.tile_pool(name="psum", bufs=8, space="PSUM") as psp,
        ):
            def pb(i, nm):
                return psp.tile([128, 512], fp32, tag=f"pb{i}", bufs=1, name=nm)

            # ---- stage inputs into SBUF ----
            wk_sb = cpool.tile([128, DCH, DH], bf16)
            wv_sb = cpool.tile([128, DCH, DH], bf16)
            x_sb = cpool.tile([128, DCH, S], bf16)
            cs_sb = cpool.tile([128, S], bf16)
            sn_sb = cpool.tile([128, S], bf16)
            wq_sb = cpool.tile([128, DCH, E], bf16)
            wo_sb = cpool.tile([128, HPG, D], bf16)
            ones8_sb = cpool.tile([128, 2, 128], f8)
            # critical path first: x chunk 0 in quarters + wk/wv chunk 0,
            # then the remaining weights (small), then the x stream which
            # paces wave 1, then cs/sn (K ropes), then wq (Q waves).
            for qt in range(2):
                nc.sync.dma_start(
                    x_sb[:, 0, qt * 1024 : (qt + 1) * 1024],
                    xT[:, 0, qt * 1024 : (qt + 1) * 1024],
                )
                if qt == 0:
                    nc.sync.dma_start(wk_sb[:, 0], wkT[:, 0])
                    nc.sync.dma_start(wv_sb[:, 0], wvT[:, 0])
            nc.sync.dma_start(x_sb[:, 1], xT[:, 1])
            nc.sync.dma_start(wk_sb[:, 1:4], wkT[:, 1:4])
            nc.sync.dma_start(wv_sb[:, 1:4], wvT[:, 1:4])
            for c in range(2, 4):
                nc.sync.dma_start(x_sb[:, c], xT[:, c])
            nc.sync.dma_start(wk_sb[:, 4:], wkT[:, 4:])
            nc.sync.dma_start(wv_sb[:, 4:], wvT[:, 4:])
            for c in range(4, DCH):
                nc.sync.dma_start(x_sb[:, c], xT[:, c])
            nc.sync.dma_start(cs_sb[:], cs[:])
            nc.sync.dma_start(sn_sb[:], sn[:])
            for p in range(4):
                nc.sync.dma_start(wq_sb[:, 4 * p : 4 * p + 4], wqT[:, 4 * p : 4 * p + 4])
            nc.sync.dma_start(ones8_sb[:], ones8[:])
            if variant == "causal":
                cm_sb = cpool.tile([128, 128], bf16)
                nc.sync.dma_start(cm_sb[:], cm[:])
            nc.sync.dma_start(wo_sb[:], woT[:])
            ones_sb = cpool.tile([128, 128], bf16)
            nc.vector.memset(ones_sb[:], 1.0 / 64)

            kT_sb = cpool.tile([128, S], bf16)
            v_sb = cpool.tile([128, STL, DH], bf16)
            qT_sb = cpool.tile([128, HPG, S], bf16)
            avT_sb = cpool.tile([128, HPG, S], bf16)

            def rope(ps, sc, out):
                # ps: [128,512] psum fp32 ([0:64)=real, [64:128)=imag parts)
                # out: [128,512] sbuf bf16 slice
                # The scalar copy releases the PSUM bank in ~0.7us (the full
                # rope took ~2.8us of DVE, stalling whoever reuses the bank);
                # bf16 muls run at 2x DVE rate and the add/sub go to GpSimd.
                lo, hi = sc * 512, (sc + 1) * 512
                m1 = tpool.tile([128, 512], fp32, tag="rope_m1", bufs=3, name="m1")
                nc.vector.tensor_mul(m1[:], ps[:], cs_sb[:, lo:hi])
                nc.vector.tensor_mul(ps[:], ps[:], sn_sb[:, lo:hi])
                nc.vector.tensor_sub(out[0:64], m1[0:64], ps[64:128])
                nc.vector.tensor_add(out[64:128], ps[0:64], m1[64:128])

            def k_mm(ps, sc, c, **fl):
                nc.tensor.matmul(
                    ps[:], wk_sb[:, c], x_sb[:, c, sc * 512 : (sc + 1) * 512], **fl
                )

            def q_mm(ps, h, sc, c, **fl):
                nc.tensor.matmul(
                    ps[:], wq_sb[:, c, h * 128 : (h + 1) * 128],
                    x_sb[:, c, sc * 512 : (sc + 1) * 512], **fl,
                )

            def v_mm(ps, st, c, **fl):
                nc.tensor.matmul(
                    ps[:, 0:128], x_sb[:, c, st * 128 : (st + 1) * 128],
                    wv_sb[:, c], **fl,
                )

            def v_copy(ps, st):
                # scalar engine: idle during projections, and keeps the bank
                # release out of the DVE rope queue
                nc.scalar.copy(v_sb[:, st], ps[:, 0:128])

            # ---- wave 1: K(sc0-3) on banks 0-3 + V(st0-3) on banks 4-7,
            # strictly chunk-ordered (paced by the x DMA stream); V runs two
            # chunks behind K so the first V matmul never heads the PE queue
            # before its weights land.
            kps = [pb(4 + i, f"kp{i}") for i in range(4)]
            vps1 = [pb(i, f"vp{i}") for i in range(4)]
            for c in range(DCH + 2):
                if c < DCH:
                    for sc in range(4):
                        k_mm(kps[sc], sc, c, start=(c == 0), stop=(c == DCH - 1))
                if c >= 2:
                    cv = c - 2
                    for g in range(4):
                        v_mm(vps1[g], g, cv, start=(cv == 0), stop=(cv == DCH - 1))
            for g in range(4):
                v_copy(vps1[g], g)
            for sc in range(4):
                rope(kps[sc], sc, kT_sb[:, sc * 512 : (sc + 1) * 512])

            # ---- wave 1.5: V(st4-9) group-major (sized to cover the K rope
            # drain; V4-7 land on the fast-released V banks, V8/V9 on K banks
            # whose ropes finish just in time)
            for g in range(6):
                vp = pb(g, f"vq{g}")
                for c in range(DCH):
                    v_mm(vp, 4 + g, c, start=(c == 0), stop=(c == DCH - 1))
                v_copy(vp, 4 + g)

            # ---- waves 2-5: Q(h) on banks 0-3 (+ V group quartet on 4-7),
            # with staggered tails so ropes overlap the next group's matmuls.
            def qv_wave(h, vsts):
                # phase A: chunks 0-7 round-robin; phase B: each Q group's
                # chunks 8-15 back-to-back + its rope, so the four ropes
                # stagger ~1.7us apart on the DVE and the early ones finish
                # BEFORE the wave ends (the following block's av/den reuse
                # their banks immediately).
                qps = [pb(i, f"qp{h}_{i}") for i in range(4)]
                vps = [pb(4 + g, f"vw{h}_{g}") for g in range(len(vsts))]
                for g, st in enumerate(vsts):
                    v_mm(vps[g], st, 0, start=True, stop=False)
                for s in range(8):
                    for sc in range(4):
                        q_mm(qps[sc], h, sc, s, start=(s == 0), stop=False)
                    for g, st in enumerate(vsts):
                        v_mm(vps[g], st, s + 1, start=False, stop=False)
                for i, sc in enumerate(range(4)):
                    for s in range(8, DCH):
                        q_mm(qps[sc], h, sc, s, start=False, stop=(s == DCH - 1))
                    rope(qps[sc], sc, qT_sb[:, h, sc * 512 : (sc + 1) * 512])
                    if i < len(vsts):
                        st = vsts[i]
                        for s in range(9, DCH):
                            v_mm(vps[i], st, s, start=False, stop=(s == DCH - 1))
                        v_copy(vps[i], st)

            def attn_block(h, qc, filler=None):
                # Software-pipelined: the PE stream interleaves score(j) with
                # av(j-LAG) and den members, so the scalar engine's exp
                # throughput (~540ns/tile vs 213ns/matmul) is hidden behind
                # the av/den matmuls instead of stalling the in-order PE.
                lo, hi = qc * 512, (qc + 1) * 512
                if variant == "causal":
                    diag = list(range(4 * qc, 4 * qc + 4))
                    full = list(range(4 * qc))
                else:
                    diag = []
                    full = list(range(STL))
                kts = diag + full          # scores order: diagonals first
                n = len(kts)
                LAG = 3
                qoff = {}
                pts = {}
                p8ts = []
                av = pb(0, "av")
                den = pb(1, "den")
                # den member emitters: (ready_pos, fn(start, stop))
                den_members = []
                if variant == "generic" or not USE_DR:
                    for j, kt in enumerate(kts):
                        def dfn(kt=kt):
                            def f(st, sp):
                                qo = qoff[kt]
                                nc.tensor.matmul(
                                    den[:, qo:512], ones_sb[:],
                                    pts[kt][:, qo:512], start=st, stop=sp,
                                )
                            return f
                        den_members.append((j, dfn()))
                else:
                    for dj, kt in enumerate(diag):
                        def dfn(kt=kt):
                            def f(st, sp):
                                qo = qoff[kt]
                                nc.tensor.matmul(
                                    den[:, qo:512], ones_sb[:],
                                    pts[kt][:, qo:512], start=st, stop=sp,
                                )
                            return f
                        den_members.append((dj, dfn()))
                    for p in range(len(full) // 2):
                        def dfn(p=p):
                            def f(st, sp):
                                nc.tensor.matmul(
                                    den[:], ones8_sb[:], p8ts[p][:],
                                    start=st, stop=sp, perf_mode=DR,
                                )
                            return f
                        den_members.append((len(diag) + 2 * p + 1, dfn()))
                nden = len(den_members)
                den_next = 0

                def emit_score(j):
                    kt = kts[j]
                    t = kt - 4 * qc
                    qo = 128 * t if (variant == "causal" and t >= 0) else 0
                    qoff[kt] = qo
                    sps = pb(4 + j % 4, "sps")
                    nc.tensor.matmul(
                        sps[:, qo:512], kT_sb[:, kt * 128 : (kt + 1) * 128],
                        qT_sb[:, h, lo + qo : hi], start=True, stop=True,
                    )
                    pt = ptpool.tile([128, 512], bf16, tag="pt", bufs=14, name="pt")
                    pts[kt] = pt
                    if variant == "generic":
                        mt = tpool.tile([128, 512], fp32, tag="mt", bufs=2, name="mt")
                        nc.sync.dma_start(mt[:], mT[:, kt, lo:hi])
                        stt = tpool.tile([128, 512], fp32, tag="stt", bufs=2, name="stt")
                        nc.vector.scalar_tensor_tensor(
                            stt[:], sps[:], SCALE, mt[:],
                            op0=mybir.AluOpType.mult, op1=mybir.AluOpType.add,
                        )
                        nc.scalar.activation(pt[:], stt[:], EXP)
                    else:
                        nc.scalar.activation(
                            pt[:, qo:512], sps[:, qo:512], EXP, scale=SCALE
                        )
                    if variant == "causal" and t >= 0:
                        # only the first 128 columns of the trimmed region are
                        # partially masked; run on the otherwise-idle GpSimd so
                        # DVE backlog can't delay the den/av matmuls
                        nc.gpsimd.tensor_mul(
                            pt[:, qo : qo + 128], pt[:, qo : qo + 128], cm_sb[:]
                        )
                    if USE_DR and kt in full and variant != "generic":
                        # fp8 copy for the DoubleRow denominator pass;
                        # scale 1/64 + clamp keeps exp() within e4m3 range
                        if kt % 2 == 0:
                            p8t = p8pool.tile(
                                [128, 2, 512], f8, tag="p8", bufs=8, name="p8"
                            )
                            p8ts.append(p8t)
                        nc.vector.tensor_scalar(
                            p8ts[-1][:, kt % 2], pt[:], 1.0 / 64, 240.0,
                            op0=mybir.AluOpType.mult, op1=mybir.AluOpType.min,
                        )

                for j in range(n + LAG):
                    if j < n:
                        emit_score(j)
                    jj = j - LAG
                    if jj < 0:
                        continue
                    kt = kts[jj]
                    qo = qoff[kt]
                    nc.tensor.matmul(
                        av[:, qo:512], v_sb[:, kt], pts[kt][:, qo:512],
                        start=(jj == 0), stop=(jj == n - 1),
                    )
                    while den_next < nden and den_members[den_next][0] <= jj:
                        den_members[den_next][1](
                            den_next == 0, den_next == nden - 1
                        )
                        den_next += 1
                    if filler is not None and jj % 2 == 1:
                        f = next(filler, None)
                        if f is not None:
                            f()
                rcp = tpool.tile([128, 512], fp32, tag="rcp", bufs=3, name="rcp")
                nc.vector.reciprocal_approx_fast(out=rcp[:], in_=den[:])
                nc.vector.scalar_tensor_tensor(
                    avT_sb[:, h, lo:hi], av[:], 1.0 / 64, rcp[:],
                    op0=mybir.AluOpType.mult, op1=mybir.AluOpType.mult,
                )

            def oproj_groups(qc):
                last = qc == SCH - 1
                for sti in range(4):
                    st = qc * 4 + sti
                    orow = None
                    if not last:
                        orow = opool.tile(
                            [128, 2048], bf16, tag="orow", bufs=2, name="orow"
                        )
                    for dc in range(4):
                        def emit(st=st, dc=dc, sti=sti, orow=orow):
                            ops = pb(2 + (sti * 4 + dc) % 2, "ops")
                            for h in range(HPG):
                                nc.tensor.matmul(
                                    ops[:], avT_sb[:, h, st * 128 : (st + 1) * 128],
                                    wo_sb[:, h, dc * 512 : (dc + 1) * 512],
                                    start=(h == 0), stop=(h == HPG - 1),
                                )
                            if last:
                                ot = opool.tile(
                                    [128, 512], bf16, tag="ot", bufs=4, name="ot"
                                )
                                if sti >= 2:
                                    # split the drain across both engines
                                    nc.scalar.copy(ot[:, 0:256], ops[:, 0:256])
                                    nc.vector.tensor_copy(
                                        ot[:, 256:512], ops[:, 256:512]
                                    )
                                else:
                                    nc.vector.tensor_copy(ot[:], ops[:])
                                nc.sync.dma_start(
                                    outp[st, :, dc * 512 : (dc + 1) * 512], ot[:]
                                )
                            else:
                                dst = orow[:, dc * 512 : (dc + 1) * 512]
                                if dc == 1:
                                    nc.scalar.copy(dst, ops[:])
                                else:
                                    nc.vector.tensor_copy(dst, ops[:])
                                if dc == 3:
                                    nc.sync.dma_start(outp[st], orow[:])
                        yield emit

            qv_wave(0, [10, 11])
            attn_block(0, 0)
            qv_wave(1, [12, 13])
            attn_block(0, 1)
            qv_wave(2, [14, 15])
            attn_block(0, 2)
            qv_wave(3, [])
            attn_block(0, 3)
            for h in (1, 2, 3):
                attn_block(h, 0)
            for qc in range(1, SCH):
                filler = oproj_groups(qc - 1)
                for h in (1, 2, 3):
                    attn_block(h, qc, filler)
                for f in filler:
                    f()
            for f in oproj_groups(SCH - 1):
                f()

    nc.compile()
    return nc


def _get_nc(variant):
    if variant not in _nc_cache:
        _nc_cache[variant] = _build(variant)
    return _nc_cache[variant]


def _arrange_dT(m):
    """[r, D_contract] -> [128, D_contract//128, r]: out[p, c, i] = m[i, c*128+p]."""
    r, d = m.shape
    return np.ascontiguousarray(m.T.reshape(d // 128, 128, r).transpose(1, 0, 2))


def _pick_variant(mask):
    m = np.asarray(mask, dtype=np.float32).reshape(S, S)
    tri = np.triu(np.ones((S, S), dtype=bool), k=1)
    if np.all(m[~tri] == 0.0) and np.all(m[tri] <= -1e8):
        return "causal", m
    if np.all(m == 0.0):
        return "nomask", m
    return "generic", m


def _prep_in_maps(x, freqs_cos, freqs_sin, mask, wq, wk, wv, wo):
    x = np.asarray(x, dtype=np.float32)
    wq = np.asarray(wq, dtype=np.float32)
    wk = np.asarray(wk, dtype=np.float32)
    wv = np.asarray(wv, dtype=np.float32)
    wo = np.asarray(wo, dtype=np.float32)
    fc = np.asarray(freqs_cos, dtype=np.float32)
    fs = np.asarray(freqs_sin, dtype=np.float32)

    variant, m = _pick_variant(mask)

    # even head-dim indices (real) first, odd (imag) second
    perm = np.concatenate([np.arange(0, DH, 2), np.arange(1, DH, 2)])

    cosT = np.ascontiguousarray(fc.T)  # [64, S]
    sinT = np.ascontiguousarray(fs.T)
    cs = np.concatenate([cosT, cosT], axis=0).astype(BF16)  # [128, S]
    sn = np.concatenate([sinT, sinT], axis=0).astype(BF16)

    xT = [_arrange_dT(x[b]).astype(BF16) for b in range(B)]

    per_group = []
    for g in range(G):
        wq_g = wq[g * E : (g + 1) * E].reshape(HPG, DH, D)[:, perm, :].reshape(E, D)
        wk_g = wk[g * DH : (g + 1) * DH][perm, :]
        wv_g = wv[g * DH : (g + 1) * DH]
        wo_g = wo[:, g * E : (g + 1) * E]  # [D, E]
        woT_g = np.ascontiguousarray(
            wo_g.T.reshape(HPG, DH, D).transpose(1, 0, 2)
        )  # [128, HPG, D]
        per_group.append(
            {
                "wqT": _arrange_dT(wq_g).astype(BF16),
                "wkT": _arrange_dT(wk_g).astype(BF16),
                "wvT": _arrange_dT(wv_g).astype(BF16),
                "woT": woT_g.astype(BF16),
            }
        )

    extra = {"ones8": np.ones((128, 2, 128), dtype=F8)}
    if variant == "causal":
        p_idx = np.arange(128)[:, None]
        j_idx = np.arange(128)[None, :]
        extra["ltri"] = (p_idx <= j_idx).astype(BF16)
    elif variant == "generic":
        # maskT[k, q] = mask[q, k], arranged [128, STL, S]
        extra["maskT"] = np.ascontiguousarray(
            m.T.reshape(STL, 128, S).transpose(1, 0, 2)
        ).astype(np.float32)

    in_maps = []
    for core in range(8):
        b, g = core // G, core % G
        im = {"xT": xT[b], "cs": cs, "sn": sn}
        im.update(per_group[g])
        im.update(extra)
        in_maps.append(im)
    return in_maps, variant


def _run(inputs, trace=False, trace_cores=None):
    # always try to register the hook: bass_utils imports it whenever tracing
    # is requested (including via the BASS_TRACE env var)
    _install_profile_hook()
    from concourse.bass_utils import run_bass_kernel_spmd

    in_maps, variant = _prep_in_maps(**inputs)
    nc = _get_nc(variant)
    res = run_bass_kernel_spmd(
        nc, in_maps, core_ids=list(range(8)), trace=trace, trace_cores=trace_cores
    )
    out = np.zeros((B, S, D), dtype=np.float32)
    for core in range(8):
        b = core // G
        out[b] += res.results[core]["out"].reshape(S, D).astype(np.float32)
    return out, res


def kernel(**inputs) -> np.ndarray:
    out, _ = _run(inputs, trace=False)
    return out
